# revision 27
# baseline (speedup 1.0000x reference)
"""Causal depthwise Conv1d (K=4) for Trainium2, 8 NeuronCores.

Problem: x (B=8, L=4096, D=1024) f32, w (D, 1, 4), b (D,)
  y[n, l, d] = sum_k w[d, 0, k] * x[n, l - 3 + k, d] + b[d]   (zero pad l<0)

Sharding: data-parallel over batch — core i computes batch item i.

Per-core device kernel (PE-centric fp32r design):
  1. DMA natural [128_l, D] superblocks in (f32r view of the same bits).
  2. PE transposes 128x128 blocks -> channels-on-partitions (PSUM); DVE
     copies to SBUF tiles with a 3-column causal halo.
  3. The 4-tap MAC runs on the PE as 4 PSUM-accumulated diag matmuls:
       yt_ps[d, l] = sum_k diag(w_k)[d,d'] @ xt[d', l-3+k]
     (bf16 stationary+moving, 512-wide moving -> 1 cycle/row; f32 PSUM).
  4. ScalarE copies yt_ps -> SBUF with the per-channel bias fused,
     casting to bf16 (halves the out-transpose cost; harness gate 2e-2).
  5. PE transposes back to natural layout (bf16, 1 cycle/row);
     Pool/DVE/ScalarE copy PSUM->SBUF casting back to f32; DMA out.
"""

import sys
import types

import numpy as np

try:  # the NTFF profile hook module is absent in some containers
    import antenv.axon_hooks  # noqa: F401
except Exception:
    _stub = types.ModuleType("antenv.axon_hooks")
    _stub.get_axon_ntff_profile_hook = lambda: None
    try:
        import antenv

        sys.modules["antenv.axon_hooks"] = _stub
        antenv.axon_hooks = _stub
    except Exception:
        _pkg = types.ModuleType("antenv")
        _pkg.axon_hooks = _stub
        sys.modules["antenv"] = _pkg
        sys.modules["antenv.axon_hooks"] = _stub

import concourse.bass as bass
import concourse.bacc as bacc
import concourse.mybir as mybir
from concourse.tile import TileContext
from concourse.masks import make_identity
from concourse.bass_utils import run_bass_kernel_spmd

P = 128
B = 8
L = 4096
D = 1024
K = 4
SB = 512  # L-superblock

CFG = {
    "ident_in": "f32",    # in-transpose moving dtype ("f32": 2 cyc/row, verifier-safe)
    "yt_bf16": True,      # yt + out-transposes in bf16 (1 cyc/row)
    "stage_eng": "dve",   # xt_ps -> xt copy engine
    # final copy engine per (t,h) slot within a superblock (8 slots)
    # GpSimd cannot read PSUM on TRN2 -> only act/dve here
    "fin_pattern": ("act", "dve", "act", "dve", "act", "dve", "act", "dve"),
    "halo_eng": "pool",   # halo copies (SBUF->SBUF) engine
    "x_f32r": True,       # declare x as f32r end-to-end: 1.5 cyc/row in-transposes
    "xin_bufs": 2,        # superblock x tiles in flight
    "xt_bufs": 2,
    "yt_bufs": 3,
    "yout_bufs": 4,       # per-t y tiles
    "psin_bufs": 3,
    "psmac_bufs": 2,
    "psout_bufs": 3,
}

ALU = mybir.AluOpType


def build_conv_nc(l=L, d=D, sb=SB, cfg=CFG, reps=1):
    G = d // P
    TPB = sb // P
    NSB = l // sb
    HD = d // 2
    GH = G // 2
    f32 = mybir.dt.float32
    f32r = mybir.dt.float32r
    bf16 = mybir.dt.bfloat16

    xdt = f32r if cfg["x_f32r"] else f32
    mac_dt = bf16
    ident_in_dt = xdt if cfg["x_f32r"] else {
        "f32": f32, "f32r": f32r, "bf16": bf16
    }[cfg["ident_in"]]
    ydt = bf16 if cfg["yt_bf16"] else f32

    nc = bacc.Bacc("TRN2", target_bir_lowering=False)
    x_d = nc.dram_tensor("x", [l, d], xdt, kind="ExternalInput")
    wcols_d = nc.dram_tensor("wcols", [P, G * K], f32, kind="ExternalInput")
    bcol_d = nc.dram_tensor("bcol", [P, G], f32, kind="ExternalInput")
    y_d = nc.dram_tensor("y", [l, d], f32, kind="ExternalOutput")

    with TileContext(nc) as tc:
        with (
            tc.tile_pool(name="const", bufs=1) as constp,
            tc.tile_pool(name="xin", bufs=cfg["xin_bufs"]) as xinp,
            tc.tile_pool(name="xt", bufs=cfg["xt_bufs"]) as xtp,
            tc.tile_pool(name="yt", bufs=cfg["yt_bufs"]) as ytp,
            tc.tile_pool(name="yout", bufs=cfg["yout_bufs"]) as youtp,
            tc.tile_pool(name="ps_in", bufs=cfg["psin_bufs"], space="PSUM") as psin,
            tc.tile_pool(name="ps_mac", bufs=cfg["psmac_bufs"], space="PSUM") as psmac,
            tc.tile_pool(name="ps_out", bufs=cfg["psout_bufs"], space="PSUM") as psout,
        ):
            if ident_in_dt == f32r:
                # memset/affine_select don't take f32r tiles; build in f32
                # then cast-copy (the cast satisfies the f32r rounding rule)
                ident_f32 = constp.tile([P, P], f32)
                make_identity(nc, ident_f32)
                ident_in = constp.tile([P, P], f32r)
                nc.vector.tensor_copy(out=ident_in[:, :], in_=ident_f32[:, :])
            else:
                ident_in = constp.tile([P, P], ident_in_dt)
                make_identity(nc, ident_in)
            if ydt == ident_in_dt:
                ident_out = ident_in
            else:
                ident_out = constp.tile([P, P], ydt)
                make_identity(nc, ident_out)
            bcol = constp.tile([P, G], f32)
            nc.sync.dma_start(out=bcol, in_=bcol_d[:, :])
            zhalo = constp.tile([P, K - 1], mac_dt)
            nc.vector.memset(zhalo[:, :], 0.0)
            # build diag(w) stationaries on device: dw[:, c, :] = wcols[:, c] * I
            wcols = constp.tile([P, G * K], f32)
            nc.sync.dma_start(out=wcols, in_=wcols_d[:, :])
            ident_mac = (
                ident_out
                if ydt == mac_dt
                else (ident_in if ident_in_dt == mac_dt else None)
            )
            if ident_mac is None:
                ident_mac = constp.tile([P, P], mac_dt)
                make_identity(nc, ident_mac)
            dw = constp.tile([P, G * K, P], mac_dt)
            for c in range(G * K):
                nc.gpsimd.tensor_tensor(
                    out=dw[:, c, :],
                    in0=wcols[:, c : c + 1].broadcast_to([P, P]),
                    in1=ident_mac[:, :],
                    op=ALU.mult,
                )

            x_r = x_d[:, :].rearrange("(s t p) d -> s p t d", p=P, t=TPB)
            y_r = y_d[:, :].rearrange("(n p) d -> n p d", p=P)

            import contextlib

            loop_cm = (
                tc.For_i(0, reps, 1, hint_engines=(mybir.EngineType.PE,))
                if reps > 1
                else contextlib.nullcontext()
            )
            stage_eng = {
                "dve": nc.vector.tensor_copy,
                "act": lambda out, in_: nc.scalar.copy(out=out, in_=in_),
                "pool": nc.gpsimd.tensor_copy,
            }[cfg["stage_eng"]]
            fin_engs = {
                "dve": nc.vector.tensor_copy,
                "act": lambda out, in_: nc.scalar.copy(out=out, in_=in_),
                "pool": nc.gpsimd.tensor_copy,
            }
            halo_eng = {
                "dve": nc.vector.tensor_copy,
                "pool": nc.gpsimd.tensor_copy,
            }[cfg["halo_eng"]]
            prev_xt = [None] * G

            def emit_x_dma(s):
                # per-t slice DMAs: transposes of block t start as soon as
                # slice t lands, instead of waiting for the full superblock
                x_tile = xinp.tile([P, TPB, d], xdt)
                for t in range(TPB):
                    nc.sync.dma_start(out=x_tile[:, t, :], in_=x_r[s][:, t, :])
                return x_tile

            with loop_cm:
              x_tiles = emit_x_dma(0)
              for s in range(NSB):
                  cur_x = x_tiles
                  if s + 1 < NSB:
                      x_tiles = emit_x_dma(s + 1)

                  yts = []

                  def emit_transpose_stage(g):
                      # transpose-in: [128_l, 128_d] blocks -> [128_d, SB_l] psum
                      xt_ps = psin.tile([P, sb], xdt)
                      for t in range(TPB):
                          nc.tensor.transpose(
                              xt_ps[:, t * P : (t + 1) * P],
                              cur_x[:, t, g * P : (g + 1) * P],
                              ident_in,
                          )
                      # haloed SBUF tile: cols [0,3) = previous superblock tail
                      xt = xtp.tile([P, K - 1 + sb], mac_dt, tag=f"xt{g}")
                      if s == 0:
                          halo_eng(out=xt[:, 0 : K - 1], in_=zhalo[:, :])
                      else:
                          halo_eng(
                              out=xt[:, 0 : K - 1],
                              in_=prev_xt[g][:, sb : sb + K - 1],
                          )
                      stage_eng(out=xt[:, K - 1 :], in_=xt_ps[:, :])
                      prev_xt[g] = xt

                  def emit_mac_bias(g):
                      # 4-tap MAC: PSUM-accumulated diag matmuls (bf16)
                      xt = prev_xt[g]
                      yt_ps = psmac.tile([P, sb], f32)
                      for k in range(K):
                          nc.tensor.matmul(
                              yt_ps[:, :],
                              dw[:, g * K + k, :],
                              xt[:, k : k + sb],
                              start=(k == 0),
                              stop=(k == K - 1),
                          )
                      yt = ytp.tile([P, sb], ydt, tag=f"yt{g}")
                      nc.scalar.activation(
                          yt[:, :],
                          yt_ps[:, :],
                          mybir.ActivationFunctionType.Identity,
                          bias=bcol[:, g : g + 1],
                          scale=1.0,
                      )
                      yts.append(yt)

                  # software-pipelined PE stream: transposes of g+1 are
                  # emitted before the MAC of g, hiding the stage-copy
                  # latency from the PE's in-order queue.
                  for g in range(G):
                      emit_transpose_stage(g)
                      if g >= 1:
                          emit_mac_bias(g - 1)
                  emit_mac_bias(G - 1)

                  # transpose-out per (t, d-half) + copy; store per t
                  for t in range(TPB):
                      y_tile = youtp.tile([P, d], f32)
                      for h in range(2):
                          y_ps = psout.tile([P, HD], ydt)
                          for j in range(GH):
                              g = h * GH + j
                              nc.tensor.transpose(
                                  y_ps[:, j * P : (j + 1) * P],
                                  yts[g][:, t * P : (t + 1) * P],
                                  ident_out,
                              )
                          dst = y_tile[:, h * HD : (h + 1) * HD]
                          fin_engs[cfg["fin_pattern"][t * 2 + h]](
                              out=dst, in_=y_ps[:, :]
                          )
                      nc.sync.dma_start(out=y_r[s * TPB + t], in_=y_tile)
    nc.finalize()
    return nc


def host_prep(w, b):
    w = np.asarray(w, dtype=np.float32).reshape(D, K)
    b = np.asarray(b, dtype=np.float32).reshape(D)
    G = D // P
    wcols = np.empty((P, G * K), dtype=np.float32)
    bcol = np.empty((P, G), dtype=np.float32)
    for g in range(G):
        bcol[:, g] = b[g * P : (g + 1) * P]
        for k in range(K):
            wcols[:, g * K + k] = w[g * P : (g + 1) * P, k]
    return {"wcols": wcols, "bcol": bcol}


_NC_CACHE = {}


def _get_nc():
    key = (L, D, SB)
    if key not in _NC_CACHE:
        _NC_CACHE[key] = build_conv_nc()
    return _NC_CACHE[key]


def kernel(x, w, b, _trace=False):
    x = np.asarray(x, dtype=np.float32)
    assert x.shape == (B, L, D), x.shape
    consts = host_prep(w, b)
    nc = _get_nc()
    in_maps = [{"x": np.ascontiguousarray(x[i]), **consts} for i in range(B)]
    res = run_bass_kernel_spmd(nc, in_maps, core_ids=list(range(B)), trace=_trace)
    y = np.stack([res.results[i]["y"] for i in range(B)], axis=0)
    if _trace:
        return y, res
    return y


# revision 28
# speedup vs baseline: 1.0564x; 1.0564x over previous
"""Causal depthwise Conv1d (K=4) for Trainium2, 8 NeuronCores.

Problem: x (B=8, L=4096, D=1024) f32, w (D, 1, 4), b (D,)
  y[n, l, d] = sum_k w[d, 0, k] * x[n, l - 3 + k, d] + b[d]   (zero pad l<0)

Sharding: data-parallel over batch — core i computes batch item i.

Per-core device kernel (PE-centric fp32r design):
  1. DMA natural [128_l, D] superblocks in (f32r view of the same bits).
  2. PE transposes 128x128 blocks -> channels-on-partitions (PSUM); DVE
     copies to SBUF tiles with a 3-column causal halo.
  3. The 4-tap MAC runs on the PE as 4 PSUM-accumulated diag matmuls:
       yt_ps[d, l] = sum_k diag(w_k)[d,d'] @ xt[d', l-3+k]
     (bf16 stationary+moving, 512-wide moving -> 1 cycle/row; f32 PSUM).
  4. ScalarE copies yt_ps -> SBUF with the per-channel bias fused,
     casting to bf16 (halves the out-transpose cost; harness gate 2e-2).
  5. PE transposes back to natural layout (bf16, 1 cycle/row);
     Pool/DVE/ScalarE copy PSUM->SBUF casting back to f32; DMA out.
"""

import sys
import types

import numpy as np

try:  # the NTFF profile hook module is absent in some containers
    import antenv.axon_hooks  # noqa: F401
except Exception:
    _stub = types.ModuleType("antenv.axon_hooks")
    _stub.get_axon_ntff_profile_hook = lambda: None
    try:
        import antenv

        sys.modules["antenv.axon_hooks"] = _stub
        antenv.axon_hooks = _stub
    except Exception:
        _pkg = types.ModuleType("antenv")
        _pkg.axon_hooks = _stub
        sys.modules["antenv"] = _pkg
        sys.modules["antenv.axon_hooks"] = _stub

import concourse.bass as bass
import concourse.bacc as bacc
import concourse.mybir as mybir
from concourse.tile import TileContext
from concourse.masks import make_identity
from concourse.bass_utils import run_bass_kernel_spmd

P = 128
B = 8
L = 4096
D = 1024
K = 4
SB = 512  # L-superblock

CFG = {
    "ident_in": "f32",    # in-transpose moving dtype ("f32": 2 cyc/row, verifier-safe)
    "yt_bf16": True,      # yt + out-transposes in bf16 (1 cyc/row)
    "stage_eng": "dve",   # xt_ps -> xt copy engine
    # final copy engine per (t,h) slot within a superblock (8 slots)
    # GpSimd cannot read PSUM on TRN2 -> only act/dve here
    "fin_pattern": ("act", "dve", "act", "dve", "act", "dve", "act", "dve"),
    "halo_eng": "pool",   # halo copies (SBUF->SBUF) engine
    "x_f32r": True,       # declare x as f32r end-to-end: 1.5 cyc/row in-transposes
    "xin_bufs": 2,        # superblock x tiles in flight
    "xt_bufs": 2,
    "yt_bufs": 3,
    "yout_bufs": 4,       # per-t y tiles
    "psin_bufs": 3,
    "psmac_bufs": 2,
    "psout_bufs": 3,
}

ALU = mybir.AluOpType


def build_conv_nc(l=L, d=D, sb=SB, cfg=CFG, reps=1):
    G = d // P
    TPB = sb // P
    NSB = l // sb
    HD = d // 2
    GH = G // 2
    f32 = mybir.dt.float32
    f32r = mybir.dt.float32r
    bf16 = mybir.dt.bfloat16

    xdt = f32r if cfg["x_f32r"] else f32
    mac_dt = bf16
    ident_in_dt = xdt if cfg["x_f32r"] else {
        "f32": f32, "f32r": f32r, "bf16": bf16
    }[cfg["ident_in"]]
    ydt = bf16 if cfg["yt_bf16"] else f32

    nc = bacc.Bacc("TRN2", target_bir_lowering=False)
    x_d = nc.dram_tensor("x", [l, d], xdt, kind="ExternalInput")
    wcols_d = nc.dram_tensor("wcols", [P, G * K], f32, kind="ExternalInput")
    bcol_d = nc.dram_tensor("bcol", [P, G], f32, kind="ExternalInput")
    y_d = nc.dram_tensor("y", [l, d], f32, kind="ExternalOutput")

    with TileContext(nc) as tc:
        with (
            tc.tile_pool(name="const", bufs=1) as constp,
            tc.tile_pool(name="xin", bufs=cfg["xin_bufs"]) as xinp,
            tc.tile_pool(name="xt", bufs=cfg["xt_bufs"]) as xtp,
            tc.tile_pool(name="yt", bufs=cfg["yt_bufs"]) as ytp,
            tc.tile_pool(name="yout", bufs=cfg["yout_bufs"]) as youtp,
            tc.tile_pool(name="ps_in", bufs=cfg["psin_bufs"], space="PSUM") as psin,
            tc.tile_pool(name="ps_mac", bufs=cfg["psmac_bufs"], space="PSUM") as psmac,
            tc.tile_pool(name="ps_out", bufs=cfg["psout_bufs"], space="PSUM") as psout,
        ):
            if ident_in_dt == f32r:
                # memset/affine_select don't take f32r tiles; build in f32
                # then cast-copy (the cast satisfies the f32r rounding rule)
                ident_f32 = constp.tile([P, P], f32)
                make_identity(nc, ident_f32)
                ident_in = constp.tile([P, P], f32r)
                nc.vector.tensor_copy(out=ident_in[:, :], in_=ident_f32[:, :])
            else:
                ident_in = constp.tile([P, P], ident_in_dt)
                make_identity(nc, ident_in)
            if ydt == ident_in_dt:
                ident_out = ident_in
            else:
                ident_out = constp.tile([P, P], ydt)
                make_identity(nc, ident_out)
            bcol = constp.tile([P, G], f32)
            nc.sync.dma_start(out=bcol, in_=bcol_d[:, :])
            zhalo = constp.tile([P, K - 1], mac_dt)
            nc.vector.memset(zhalo[:, :], 0.0)
            # build diag(w) stationaries on device: dw[:, c, :] = wcols[:, c] * I
            wcols = constp.tile([P, G * K], f32)
            nc.sync.dma_start(out=wcols, in_=wcols_d[:, :])
            ident_mac = (
                ident_out
                if ydt == mac_dt
                else (ident_in if ident_in_dt == mac_dt else None)
            )
            if ident_mac is None:
                ident_mac = constp.tile([P, P], mac_dt)
                make_identity(nc, ident_mac)
            dw = constp.tile([P, G * K, P], mac_dt)
            for c in range(G * K):
                nc.gpsimd.tensor_tensor(
                    out=dw[:, c, :],
                    in0=wcols[:, c : c + 1].broadcast_to([P, P]),
                    in1=ident_mac[:, :],
                    op=ALU.mult,
                )

            x_r = x_d[:, :].rearrange("(s t p) d -> s p t d", p=P, t=TPB)
            y_r = y_d[:, :].rearrange("(n p) d -> n p d", p=P)

            import contextlib

            loop_cm = (
                tc.For_i(0, reps, 1, hint_engines=(mybir.EngineType.PE,))
                if reps > 1
                else contextlib.nullcontext()
            )
            stage_eng = {
                "dve": nc.vector.tensor_copy,
                "act": lambda out, in_: nc.scalar.copy(out=out, in_=in_),
                "pool": nc.gpsimd.tensor_copy,
            }[cfg["stage_eng"]]
            fin_engs = {
                "dve": nc.vector.tensor_copy,
                "act": lambda out, in_: nc.scalar.copy(out=out, in_=in_),
                "pool": nc.gpsimd.tensor_copy,
            }
            halo_eng = {
                "dve": nc.vector.tensor_copy,
                "pool": nc.gpsimd.tensor_copy,
            }[cfg["halo_eng"]]
            prev_xt = [None] * G

            def emit_x_dma(s):
                x_tile = xinp.tile([P, TPB, d], xdt)
                if s == 0:
                    # per-t slices so the first transposes start ~4us sooner
                    for t in range(TPB):
                        nc.sync.dma_start(
                            out=x_tile[:, t, :], in_=x_r[s][:, t, :]
                        )
                else:
                    nc.sync.dma_start(out=x_tile, in_=x_r[s])
                return x_tile

            with loop_cm:
              x_tiles = emit_x_dma(0)
              for s in range(NSB):
                  cur_x = x_tiles
                  if s + 1 < NSB:
                      x_tiles = emit_x_dma(s + 1)

                  yts = []

                  def emit_transpose_stage(g):
                      # transpose-in: [128_l, 128_d] blocks -> [128_d, SB_l] psum
                      xt_ps = psin.tile([P, sb], xdt)
                      for t in range(TPB):
                          nc.tensor.transpose(
                              xt_ps[:, t * P : (t + 1) * P],
                              cur_x[:, t, g * P : (g + 1) * P],
                              ident_in,
                          )
                      # haloed SBUF tile: cols [0,3) = previous superblock tail
                      xt = xtp.tile([P, K - 1 + sb], mac_dt, tag=f"xt{g}")
                      if s == 0:
                          halo_eng(out=xt[:, 0 : K - 1], in_=zhalo[:, :])
                      else:
                          halo_eng(
                              out=xt[:, 0 : K - 1],
                              in_=prev_xt[g][:, sb : sb + K - 1],
                          )
                      stage_eng(out=xt[:, K - 1 :], in_=xt_ps[:, :])
                      prev_xt[g] = xt

                  def emit_mac_bias(g):
                      # 4-tap MAC: PSUM-accumulated diag matmuls (bf16)
                      xt = prev_xt[g]
                      yt_ps = psmac.tile([P, sb], f32)
                      for k in range(K):
                          nc.tensor.matmul(
                              yt_ps[:, :],
                              dw[:, g * K + k, :],
                              xt[:, k : k + sb],
                              start=(k == 0),
                              stop=(k == K - 1),
                          )
                      yt = ytp.tile([P, sb], ydt, tag=f"yt{g}")
                      nc.scalar.activation(
                          yt[:, :],
                          yt_ps[:, :],
                          mybir.ActivationFunctionType.Identity,
                          bias=bcol[:, g : g + 1],
                          scale=1.0,
                      )
                      yts.append(yt)

                  # software-pipelined PE stream: transposes of g+1 are
                  # emitted before the MAC of g, hiding the stage-copy
                  # latency from the PE's in-order queue.
                  for g in range(G):
                      emit_transpose_stage(g)
                      if g >= 1:
                          emit_mac_bias(g - 1)
                  emit_mac_bias(G - 1)

                  # transpose-out per (t, d-half) + copy; store per t
                  for t in range(TPB):
                      y_tile = youtp.tile([P, d], f32)
                      for h in range(2):
                          y_ps = psout.tile([P, HD], ydt)
                          for j in range(GH):
                              g = h * GH + j
                              nc.tensor.transpose(
                                  y_ps[:, j * P : (j + 1) * P],
                                  yts[g][:, t * P : (t + 1) * P],
                                  ident_out,
                              )
                          dst = y_tile[:, h * HD : (h + 1) * HD]
                          fin_engs[cfg["fin_pattern"][t * 2 + h]](
                              out=dst, in_=y_ps[:, :]
                          )
                      nc.sync.dma_start(out=y_r[s * TPB + t], in_=y_tile)
    nc.finalize()
    return nc


def host_prep(w, b):
    w = np.asarray(w, dtype=np.float32).reshape(D, K)
    b = np.asarray(b, dtype=np.float32).reshape(D)
    G = D // P
    wcols = np.empty((P, G * K), dtype=np.float32)
    bcol = np.empty((P, G), dtype=np.float32)
    for g in range(G):
        bcol[:, g] = b[g * P : (g + 1) * P]
        for k in range(K):
            wcols[:, g * K + k] = w[g * P : (g + 1) * P, k]
    return {"wcols": wcols, "bcol": bcol}


_NC_CACHE = {}


def _get_nc():
    key = (L, D, SB)
    if key not in _NC_CACHE:
        _NC_CACHE[key] = build_conv_nc()
    return _NC_CACHE[key]


def kernel(x, w, b, _trace=False):
    x = np.asarray(x, dtype=np.float32)
    assert x.shape == (B, L, D), x.shape
    consts = host_prep(w, b)
    nc = _get_nc()
    in_maps = [{"x": np.ascontiguousarray(x[i]), **consts} for i in range(B)]
    res = run_bass_kernel_spmd(nc, in_maps, core_ids=list(range(B)), trace=_trace)
    y = np.stack([res.results[i]["y"] for i in range(B)], axis=0)
    if _trace:
        return y, res
    return y


# revision 29
# speedup vs baseline: 1.0638x; 1.0071x over previous
"""Causal depthwise Conv1d (K=4) for Trainium2, 8 NeuronCores.

Problem: x (B=8, L=4096, D=1024) f32, w (D, 1, 4), b (D,)
  y[n, l, d] = sum_k w[d, 0, k] * x[n, l - 3 + k, d] + b[d]   (zero pad l<0)

Sharding: data-parallel over batch — core i computes batch item i.

Per-core device kernel (PE-centric fp32r design):
  1. DMA natural [128_l, D] superblocks in (f32r view of the same bits).
  2. PE transposes 128x128 blocks -> channels-on-partitions (PSUM); DVE
     copies to SBUF tiles with a 3-column causal halo.
  3. The 4-tap MAC runs on the PE as 4 PSUM-accumulated diag matmuls:
       yt_ps[d, l] = sum_k diag(w_k)[d,d'] @ xt[d', l-3+k]
     (bf16 stationary+moving, 512-wide moving -> 1 cycle/row; f32 PSUM).
  4. ScalarE copies yt_ps -> SBUF with the per-channel bias fused,
     casting to bf16 (halves the out-transpose cost; harness gate 2e-2).
  5. PE transposes back to natural layout (bf16, 1 cycle/row);
     Pool/DVE/ScalarE copy PSUM->SBUF casting back to f32; DMA out.
"""

import sys
import types

import numpy as np

try:  # the NTFF profile hook module is absent in some containers
    import antenv.axon_hooks  # noqa: F401
except Exception:
    _stub = types.ModuleType("antenv.axon_hooks")
    _stub.get_axon_ntff_profile_hook = lambda: None
    try:
        import antenv

        sys.modules["antenv.axon_hooks"] = _stub
        antenv.axon_hooks = _stub
    except Exception:
        _pkg = types.ModuleType("antenv")
        _pkg.axon_hooks = _stub
        sys.modules["antenv"] = _pkg
        sys.modules["antenv.axon_hooks"] = _stub

import concourse.bass as bass
import concourse.bacc as bacc
import concourse.mybir as mybir
from concourse.tile import TileContext
from concourse.masks import make_identity
from concourse.bass_utils import run_bass_kernel_spmd

P = 128
B = 8
L = 4096
D = 1024
K = 4
SB = 512  # L-superblock

CFG = {
    "ident_in": "f32",    # in-transpose moving dtype ("f32": 2 cyc/row, verifier-safe)
    "yt_bf16": True,      # yt + out-transposes in bf16 (1 cyc/row)
    "stage_eng": "dve",   # xt_ps -> xt copy engine
    # final copy engine per (t,h) slot within a superblock (8 slots)
    # GpSimd cannot read PSUM on TRN2 -> only act/dve here
    "fin_pattern": ("act", "dve", "act", "dve", "act", "dve", "act", "dve"),
    "halo_eng": "pool",   # halo copies (SBUF->SBUF) engine
    "x_f32r": True,       # declare x as f32r end-to-end: 1.5 cyc/row in-transposes
    "xin_bufs": 2,        # superblock x tiles in flight
    "xt_bufs": 2,
    "yt_bufs": 3,
    "yout_bufs": 4,       # per-t y tiles
    "psin_bufs": 4,
    "psmac_bufs": 2,
    "psout_bufs": 2,
}

ALU = mybir.AluOpType


def build_conv_nc(l=L, d=D, sb=SB, cfg=CFG, reps=1):
    G = d // P
    TPB = sb // P
    NSB = l // sb
    HD = d // 2
    GH = G // 2
    f32 = mybir.dt.float32
    f32r = mybir.dt.float32r
    bf16 = mybir.dt.bfloat16

    xdt = f32r if cfg["x_f32r"] else f32
    mac_dt = bf16
    ident_in_dt = xdt if cfg["x_f32r"] else {
        "f32": f32, "f32r": f32r, "bf16": bf16
    }[cfg["ident_in"]]
    ydt = bf16 if cfg["yt_bf16"] else f32

    nc = bacc.Bacc("TRN2", target_bir_lowering=False)
    x_d = nc.dram_tensor("x", [l, d], xdt, kind="ExternalInput")
    wcols_d = nc.dram_tensor("wcols", [P, G * K], f32, kind="ExternalInput")
    bcol_d = nc.dram_tensor("bcol", [P, G], f32, kind="ExternalInput")
    y_d = nc.dram_tensor("y", [l, d], f32, kind="ExternalOutput")

    with TileContext(nc) as tc:
        with (
            tc.tile_pool(name="const", bufs=1) as constp,
            tc.tile_pool(name="xin", bufs=cfg["xin_bufs"]) as xinp,
            tc.tile_pool(name="xt", bufs=cfg["xt_bufs"]) as xtp,
            tc.tile_pool(name="yt", bufs=cfg["yt_bufs"]) as ytp,
            tc.tile_pool(name="yout", bufs=cfg["yout_bufs"]) as youtp,
            tc.tile_pool(name="ps_in", bufs=cfg["psin_bufs"], space="PSUM") as psin,
            tc.tile_pool(name="ps_mac", bufs=cfg["psmac_bufs"], space="PSUM") as psmac,
            tc.tile_pool(name="ps_out", bufs=cfg["psout_bufs"], space="PSUM") as psout,
        ):
            if ident_in_dt == f32r:
                # memset/affine_select don't take f32r tiles; build in f32
                # then cast-copy (the cast satisfies the f32r rounding rule)
                ident_f32 = constp.tile([P, P], f32)
                make_identity(nc, ident_f32)
                ident_in = constp.tile([P, P], f32r)
                nc.vector.tensor_copy(out=ident_in[:, :], in_=ident_f32[:, :])
            else:
                ident_in = constp.tile([P, P], ident_in_dt)
                make_identity(nc, ident_in)
            if ydt == ident_in_dt:
                ident_out = ident_in
            else:
                ident_out = constp.tile([P, P], ydt)
                make_identity(nc, ident_out)
            bcol = constp.tile([P, G], f32)
            nc.sync.dma_start(out=bcol, in_=bcol_d[:, :])
            zhalo = constp.tile([P, K - 1], mac_dt)
            nc.vector.memset(zhalo[:, :], 0.0)
            # build diag(w) stationaries on device: dw[:, c, :] = wcols[:, c] * I
            wcols = constp.tile([P, G * K], f32)
            nc.sync.dma_start(out=wcols, in_=wcols_d[:, :])
            ident_mac = (
                ident_out
                if ydt == mac_dt
                else (ident_in if ident_in_dt == mac_dt else None)
            )
            if ident_mac is None:
                ident_mac = constp.tile([P, P], mac_dt)
                make_identity(nc, ident_mac)
            dw = constp.tile([P, G * K, P], mac_dt)
            for c in range(G * K):
                nc.gpsimd.tensor_tensor(
                    out=dw[:, c, :],
                    in0=wcols[:, c : c + 1].broadcast_to([P, P]),
                    in1=ident_mac[:, :],
                    op=ALU.mult,
                )

            x_r = x_d[:, :].rearrange("(s t p) d -> s p t d", p=P, t=TPB)
            y_r = y_d[:, :].rearrange("(n p) d -> n p d", p=P)

            import contextlib

            loop_cm = (
                tc.For_i(0, reps, 1, hint_engines=(mybir.EngineType.PE,))
                if reps > 1
                else contextlib.nullcontext()
            )
            stage_eng = {
                "dve": nc.vector.tensor_copy,
                "act": lambda out, in_: nc.scalar.copy(out=out, in_=in_),
                "pool": nc.gpsimd.tensor_copy,
            }[cfg["stage_eng"]]
            fin_engs = {
                "dve": nc.vector.tensor_copy,
                "act": lambda out, in_: nc.scalar.copy(out=out, in_=in_),
                "pool": nc.gpsimd.tensor_copy,
            }
            halo_eng = {
                "dve": nc.vector.tensor_copy,
                "pool": nc.gpsimd.tensor_copy,
            }[cfg["halo_eng"]]
            prev_xt = [None] * G

            def emit_x_dma(s):
                x_tile = xinp.tile([P, TPB, d], xdt)
                if s == 0:
                    # per-t slices so the first transposes start ~4us sooner
                    for t in range(TPB):
                        nc.sync.dma_start(
                            out=x_tile[:, t, :], in_=x_r[s][:, t, :]
                        )
                else:
                    nc.sync.dma_start(out=x_tile, in_=x_r[s])
                return x_tile

            with loop_cm:
              x_tiles = emit_x_dma(0)
              for s in range(NSB):
                  cur_x = x_tiles
                  if s + 1 < NSB:
                      x_tiles = emit_x_dma(s + 1)

                  yts = []

                  def emit_transpose_stage(g):
                      # transpose-in: [128_l, 128_d] blocks -> [128_d, SB_l] psum
                      xt_ps = psin.tile([P, sb], xdt)
                      for t in range(TPB):
                          nc.tensor.transpose(
                              xt_ps[:, t * P : (t + 1) * P],
                              cur_x[:, t, g * P : (g + 1) * P],
                              ident_in,
                          )
                      # haloed SBUF tile: cols [0,3) = previous superblock tail
                      xt = xtp.tile([P, K - 1 + sb], mac_dt, tag=f"xt{g}")
                      if s == 0:
                          halo_eng(out=xt[:, 0 : K - 1], in_=zhalo[:, :])
                      else:
                          halo_eng(
                              out=xt[:, 0 : K - 1],
                              in_=prev_xt[g][:, sb : sb + K - 1],
                          )
                      stage_eng(out=xt[:, K - 1 :], in_=xt_ps[:, :])
                      prev_xt[g] = xt

                  def emit_mac_bias(g):
                      # 4-tap MAC: PSUM-accumulated diag matmuls (bf16)
                      xt = prev_xt[g]
                      yt_ps = psmac.tile([P, sb], f32)
                      for k in range(K):
                          nc.tensor.matmul(
                              yt_ps[:, :],
                              dw[:, g * K + k, :],
                              xt[:, k : k + sb],
                              start=(k == 0),
                              stop=(k == K - 1),
                          )
                      yt = ytp.tile([P, sb], ydt, tag=f"yt{g}")
                      nc.scalar.activation(
                          yt[:, :],
                          yt_ps[:, :],
                          mybir.ActivationFunctionType.Identity,
                          bias=bcol[:, g : g + 1],
                          scale=1.0,
                      )
                      yts.append(yt)

                  # software-pipelined PE stream: transposes of g+1 are
                  # emitted before the MAC of g, hiding the stage-copy
                  # latency from the PE's in-order queue.
                  for g in range(G):
                      emit_transpose_stage(g)
                      if g >= 1:
                          emit_mac_bias(g - 1)
                  emit_mac_bias(G - 1)

                  # transpose-out per (t, d-half) + copy; store per t
                  for t in range(TPB):
                      y_tile = youtp.tile([P, d], f32)
                      for h in range(2):
                          y_ps = psout.tile([P, HD], ydt)
                          for j in range(GH):
                              g = h * GH + j
                              nc.tensor.transpose(
                                  y_ps[:, j * P : (j + 1) * P],
                                  yts[g][:, t * P : (t + 1) * P],
                                  ident_out,
                              )
                          dst = y_tile[:, h * HD : (h + 1) * HD]
                          fin_engs[cfg["fin_pattern"][t * 2 + h]](
                              out=dst, in_=y_ps[:, :]
                          )
                      nc.sync.dma_start(out=y_r[s * TPB + t], in_=y_tile)
    nc.finalize()
    return nc


def host_prep(w, b):
    w = np.asarray(w, dtype=np.float32).reshape(D, K)
    b = np.asarray(b, dtype=np.float32).reshape(D)
    G = D // P
    wcols = np.empty((P, G * K), dtype=np.float32)
    bcol = np.empty((P, G), dtype=np.float32)
    for g in range(G):
        bcol[:, g] = b[g * P : (g + 1) * P]
        for k in range(K):
            wcols[:, g * K + k] = w[g * P : (g + 1) * P, k]
    return {"wcols": wcols, "bcol": bcol}


_NC_CACHE = {}


def _get_nc():
    key = (L, D, SB)
    if key not in _NC_CACHE:
        _NC_CACHE[key] = build_conv_nc()
    return _NC_CACHE[key]


def kernel(x, w, b, _trace=False):
    x = np.asarray(x, dtype=np.float32)
    assert x.shape == (B, L, D), x.shape
    consts = host_prep(w, b)
    nc = _get_nc()
    in_maps = [{"x": np.ascontiguousarray(x[i]), **consts} for i in range(B)]
    res = run_bass_kernel_spmd(nc, in_maps, core_ids=list(range(B)), trace=_trace)
    y = np.stack([res.results[i]["y"] for i in range(B)], axis=0)
    if _trace:
        return y, res
    return y


# revision 30
# speedup vs baseline: 1.1258x; 1.0583x over previous
"""Causal depthwise Conv1d (K=4) for Trainium2, 8 NeuronCores.

Problem: x (B=8, L=4096, D=1024) f32, w (D, 1, 4), b (D,)
  y[n, l, d] = sum_k w[d, 0, k] * x[n, l - 3 + k, d] + b[d]   (zero pad l<0)

Sharding: data-parallel over batch — core i computes batch item i.

Per-core device kernel (PE-centric fp32r design):
  1. DMA natural [128_l, D] superblocks in (f32r view of the same bits).
  2. PE transposes 128x128 blocks -> channels-on-partitions (PSUM); DVE
     copies to SBUF tiles with a 3-column causal halo.
  3. The 4-tap MAC runs on the PE as 4 PSUM-accumulated diag matmuls:
       yt_ps[d, l] = sum_k diag(w_k)[d,d'] @ xt[d', l-3+k]
     (bf16 stationary+moving, 512-wide moving -> 1 cycle/row; f32 PSUM).
  4. ScalarE copies yt_ps -> SBUF with the per-channel bias fused,
     casting to bf16 (halves the out-transpose cost; harness gate 2e-2).
  5. PE transposes back to natural layout (bf16, 1 cycle/row);
     Pool/DVE/ScalarE copy PSUM->SBUF casting back to f32; DMA out.
"""

import sys
import types

import numpy as np

try:  # the NTFF profile hook module is absent in some containers
    import antenv.axon_hooks  # noqa: F401
except Exception:
    _stub = types.ModuleType("antenv.axon_hooks")
    _stub.get_axon_ntff_profile_hook = lambda: None
    try:
        import antenv

        sys.modules["antenv.axon_hooks"] = _stub
        antenv.axon_hooks = _stub
    except Exception:
        _pkg = types.ModuleType("antenv")
        _pkg.axon_hooks = _stub
        sys.modules["antenv"] = _pkg
        sys.modules["antenv.axon_hooks"] = _stub

import concourse.bass as bass
import concourse.bacc as bacc
import concourse.mybir as mybir
from concourse.tile import TileContext
from concourse.masks import make_identity
from concourse.bass_utils import run_bass_kernel_spmd

P = 128
B = 8
L = 4096
D = 1024
K = 4
SB = 512  # L-superblock

CFG = {
    "ident_in": "f32",    # in-transpose moving dtype ("f32": 2 cyc/row, verifier-safe)
    "yt_bf16": True,      # yt + out-transposes in bf16 (1 cyc/row)
    "stage_eng": "dve",   # xt_ps -> xt copy engine
    # final copy engine per (t,h) slot within a superblock (8 slots)
    # GpSimd cannot read PSUM on TRN2 -> only act/dve here
    "fin_pattern": ("act",) * 8,
    "halo_eng": "pool",   # halo copies (SBUF->SBUF) engine
    "x_f32r": True,       # declare x as f32r end-to-end: 1.5 cyc/row in-transposes
    "xin_bufs": 2,        # superblock x tiles in flight
    "xt_bufs": 2,
    "yt_bufs": 3,
    "yout_bufs": 4,       # per-t y tiles
    "psin_bufs": 4,
    "psmac_bufs": 2,
    "psout_bufs": 2,
}

ALU = mybir.AluOpType


def build_conv_nc(l=L, d=D, sb=SB, cfg=CFG, reps=1):
    G = d // P
    TPB = sb // P
    NSB = l // sb
    HD = d // 2
    GH = G // 2
    f32 = mybir.dt.float32
    f32r = mybir.dt.float32r
    bf16 = mybir.dt.bfloat16

    xdt = f32r if cfg["x_f32r"] else f32
    mac_dt = bf16
    ident_in_dt = xdt if cfg["x_f32r"] else {
        "f32": f32, "f32r": f32r, "bf16": bf16
    }[cfg["ident_in"]]
    ydt = bf16 if cfg["yt_bf16"] else f32

    nc = bacc.Bacc("TRN2", target_bir_lowering=False)
    x_d = nc.dram_tensor("x", [l, d], xdt, kind="ExternalInput")
    wcols_d = nc.dram_tensor("wcols", [P, G * K], f32, kind="ExternalInput")
    bcol_d = nc.dram_tensor("bcol", [P, G], f32, kind="ExternalInput")
    y_d = nc.dram_tensor("y", [l, d], f32, kind="ExternalOutput")

    with TileContext(nc) as tc:
        with (
            tc.tile_pool(name="const", bufs=1) as constp,
            tc.tile_pool(name="xin", bufs=cfg["xin_bufs"]) as xinp,
            tc.tile_pool(name="xt", bufs=cfg["xt_bufs"]) as xtp,
            tc.tile_pool(name="yt", bufs=cfg["yt_bufs"]) as ytp,
            tc.tile_pool(name="yout", bufs=cfg["yout_bufs"]) as youtp,
            tc.tile_pool(name="ps_in", bufs=cfg["psin_bufs"], space="PSUM") as psin,
            tc.tile_pool(name="ps_mac", bufs=cfg["psmac_bufs"], space="PSUM") as psmac,
            tc.tile_pool(name="ps_out", bufs=cfg["psout_bufs"], space="PSUM") as psout,
        ):
            if ident_in_dt == f32r:
                # memset/affine_select don't take f32r tiles; build in f32
                # then cast-copy (the cast satisfies the f32r rounding rule)
                ident_f32 = constp.tile([P, P], f32)
                make_identity(nc, ident_f32)
                ident_in = constp.tile([P, P], f32r)
                nc.vector.tensor_copy(out=ident_in[:, :], in_=ident_f32[:, :])
            else:
                ident_in = constp.tile([P, P], ident_in_dt)
                make_identity(nc, ident_in)
            if ydt == ident_in_dt:
                ident_out = ident_in
            else:
                ident_out = constp.tile([P, P], ydt)
                make_identity(nc, ident_out)
            bcol = constp.tile([P, G], f32)
            nc.sync.dma_start(out=bcol, in_=bcol_d[:, :])
            zhalo = constp.tile([P, K - 1], mac_dt)
            nc.vector.memset(zhalo[:, :], 0.0)
            # build diag(w) stationaries on device: dw[:, c, :] = wcols[:, c] * I
            wcols = constp.tile([P, G * K], f32)
            nc.sync.dma_start(out=wcols, in_=wcols_d[:, :])
            ident_mac = (
                ident_out
                if ydt == mac_dt
                else (ident_in if ident_in_dt == mac_dt else None)
            )
            if ident_mac is None:
                ident_mac = constp.tile([P, P], mac_dt)
                make_identity(nc, ident_mac)
            dw = constp.tile([P, G * K, P], mac_dt)
            for c in range(G * K):
                nc.gpsimd.tensor_tensor(
                    out=dw[:, c, :],
                    in0=wcols[:, c : c + 1].broadcast_to([P, P]),
                    in1=ident_mac[:, :],
                    op=ALU.mult,
                )

            x_r = x_d[:, :].rearrange("(s t p) d -> s p t d", p=P, t=TPB)
            y_r = y_d[:, :].rearrange("(n p) d -> n p d", p=P)

            import contextlib

            loop_cm = (
                tc.For_i(0, reps, 1, hint_engines=(mybir.EngineType.PE,))
                if reps > 1
                else contextlib.nullcontext()
            )
            stage_eng = {
                "dve": nc.vector.tensor_copy,
                "act": lambda out, in_: nc.scalar.copy(out=out, in_=in_),
                "pool": nc.gpsimd.tensor_copy,
            }[cfg["stage_eng"]]
            fin_engs = {
                "dve": nc.vector.tensor_copy,
                "act": lambda out, in_: nc.scalar.copy(out=out, in_=in_),
                "pool": nc.gpsimd.tensor_copy,
            }
            halo_eng = {
                "dve": nc.vector.tensor_copy,
                "pool": nc.gpsimd.tensor_copy,
            }[cfg["halo_eng"]]
            prev_xt = [None] * G

            def emit_x_dma(s):
                x_tile = xinp.tile([P, TPB, d], xdt)
                if s == 0:
                    # per-t slices so the first transposes start ~4us sooner
                    for t in range(TPB):
                        nc.sync.dma_start(
                            out=x_tile[:, t, :], in_=x_r[s][:, t, :]
                        )
                else:
                    nc.sync.dma_start(out=x_tile, in_=x_r[s])
                return x_tile

            with loop_cm:
              x_tiles = emit_x_dma(0)
              for s in range(NSB):
                  cur_x = x_tiles
                  if s + 1 < NSB:
                      x_tiles = emit_x_dma(s + 1)

                  yts = []

                  def emit_transpose_stage(g):
                      # transpose-in: [128_l, 128_d] blocks -> [128_d, SB_l] psum
                      xt_ps = psin.tile([P, sb], xdt)
                      for t in range(TPB):
                          nc.tensor.transpose(
                              xt_ps[:, t * P : (t + 1) * P],
                              cur_x[:, t, g * P : (g + 1) * P],
                              ident_in,
                          )
                      # haloed SBUF tile: cols [0,3) = previous superblock tail
                      xt = xtp.tile([P, K - 1 + sb], mac_dt, tag=f"xt{g}")
                      if s == 0:
                          halo_eng(out=xt[:, 0 : K - 1], in_=zhalo[:, :])
                      else:
                          halo_eng(
                              out=xt[:, 0 : K - 1],
                              in_=prev_xt[g][:, sb : sb + K - 1],
                          )
                      stage_eng(out=xt[:, K - 1 :], in_=xt_ps[:, :])
                      prev_xt[g] = xt

                  def emit_mac_bias(g):
                      # 4-tap MAC: PSUM-accumulated diag matmuls (bf16)
                      xt = prev_xt[g]
                      yt_ps = psmac.tile([P, sb], f32)
                      for k in range(K):
                          nc.tensor.matmul(
                              yt_ps[:, :],
                              dw[:, g * K + k, :],
                              xt[:, k : k + sb],
                              start=(k == 0),
                              stop=(k == K - 1),
                          )
                      yt = ytp.tile([P, sb], ydt, tag=f"yt{g}")
                      nc.scalar.activation(
                          yt[:, :],
                          yt_ps[:, :],
                          mybir.ActivationFunctionType.Identity,
                          bias=bcol[:, g : g + 1],
                          scale=1.0,
                      )
                      yts.append(yt)

                  # software-pipelined PE stream: transposes of g+1 are
                  # emitted before the MAC of g, hiding the stage-copy
                  # latency from the PE's in-order queue.
                  for g in range(G):
                      emit_transpose_stage(g)
                      if g >= 1:
                          emit_mac_bias(g - 1)
                  emit_mac_bias(G - 1)

                  # transpose-out per (t, d-half) + copy; store per t
                  for t in range(TPB):
                      y_tile = youtp.tile([P, d], f32)
                      for h in range(2):
                          y_ps = psout.tile([P, HD], ydt)
                          for j in range(GH):
                              g = h * GH + j
                              nc.tensor.transpose(
                                  y_ps[:, j * P : (j + 1) * P],
                                  yts[g][:, t * P : (t + 1) * P],
                                  ident_out,
                              )
                          dst = y_tile[:, h * HD : (h + 1) * HD]
                          fin_engs[cfg["fin_pattern"][t * 2 + h]](
                              out=dst, in_=y_ps[:, :]
                          )
                      nc.sync.dma_start(out=y_r[s * TPB + t], in_=y_tile)
    nc.finalize()
    return nc


def host_prep(w, b):
    w = np.asarray(w, dtype=np.float32).reshape(D, K)
    b = np.asarray(b, dtype=np.float32).reshape(D)
    G = D // P
    wcols = np.empty((P, G * K), dtype=np.float32)
    bcol = np.empty((P, G), dtype=np.float32)
    for g in range(G):
        bcol[:, g] = b[g * P : (g + 1) * P]
        for k in range(K):
            wcols[:, g * K + k] = w[g * P : (g + 1) * P, k]
    return {"wcols": wcols, "bcol": bcol}


_NC_CACHE = {}


def _get_nc():
    key = (L, D, SB)
    if key not in _NC_CACHE:
        _NC_CACHE[key] = build_conv_nc()
    return _NC_CACHE[key]


def kernel(x, w, b, _trace=False):
    x = np.asarray(x, dtype=np.float32)
    assert x.shape == (B, L, D), x.shape
    consts = host_prep(w, b)
    nc = _get_nc()
    in_maps = [{"x": np.ascontiguousarray(x[i]), **consts} for i in range(B)]
    res = run_bass_kernel_spmd(nc, in_maps, core_ids=list(range(B)), trace=_trace)
    y = np.stack([res.results[i]["y"] for i in range(B)], axis=0)
    if _trace:
        return y, res
    return y
